# revision 24
# baseline (speedup 1.0000x reference)
"""Trainium2 Bass kernel for int4-grouped-quantized linear (GPTQ-style).

out[8192, 11008] = x[8192, 4096] @ dequant(qweight, qzeros, scales)

Sharding: column-parallel over out_features N across 8 NeuronCores.

Per core: x arrives transposed via X-bar DMA-transpose (one 2MB instruction
per 256-row chunk), W is dequantized on-chip in three column "waves" so the
PE can start consuming partially-dequantized W while the unpack stream is
still running, and the matmuls accumulate fp16 x fp16 -> fp32 PSUM.

Device W columns are nibble-plane-major (device col j*CS + c holds logical
out col c*8 + j) so the int4 unpack writes contiguously; the host permutes
`scales`/zero-points to match and un-permutes output columns.

The zero/scale rows are pre-broadcast across partitions on the host (one
[128, 2*seg] fp16 block per (wave, k-block)) so the kernel loads them with
plain contiguous HWDGE DMAs instead of slow SWDGE partition-broadcasts.
"""

import sys

sys.path.insert(0, "/opt/trn_rl_repo")

from contextlib import ExitStack

import numpy as np

import concourse.bass as bass
from concourse import bacc
import concourse.tile as tile
from concourse import mybir
from concourse.bass_utils import run_bass_kernel_spmd

AOT = mybir.AluOpType
F16, I32, F32 = mybir.dt.float16, mybir.dt.int32, mybir.dt.float32

T, K, N = 8192, 4096, 11008
NCORES = 8
NS = N // NCORES  # 1376 out cols per core
CS = NS // 8  # 172 packed int32 cols per core
G = 32  # quant groups (group size 128 == one k-block)
KB = K // 128  # 32 k-blocks
TC = 256  # t rows per x-transpose chunk
NCH = T // TC  # 32 chunks
TSUB = TC // 128  # 2 output row-blocks per chunk
SEGS = [(0, 512), (512, 512), (1024, 352)]  # N segments (PSUM bank sized)
# wave w: shift planes PLANES[w], then apply (w4-z)*s on SEGS[w]
PLANES = [(0, 3), (3, 3), (6, 2)]  # (first plane, count)
NWCH = 3  # chunks consumed seg-wise during the dequant waves
QG = 4  # k-blocks per qweight prefetch DMA


def _body(ctx, tc, xd, qwd, zsd, outd):
    nc = tc.nc
    qpool = ctx.enter_context(tc.tile_pool(name="qwp", bufs=KB // QG))
    stpool = ctx.enter_context(tc.tile_pool(name="stage", bufs=3))
    wpool = ctx.enter_context(tc.tile_pool(name="w", bufs=KB))
    zpool = ctx.enter_context(tc.tile_pool(name="zs", bufs=4))
    xtpool = ctx.enter_context(tc.tile_pool(name="xt", bufs=NWCH + 1))
    pspool = ctx.enter_context(tc.tile_pool(name="ps", bufs=8, space="PSUM"))
    opool = ctx.enter_context(tc.tile_pool(name="o", bufs=8))

    # resident packed weights (grouped DMAs) and fp16 W tiles
    qw_gs = []
    for g in range(KB // QG):
        qw_g = qpool.tile([128, QG, CS], I32, name=f"qwg{g}", tag="qw")
        nc.gpsimd.dma_start(
            qw_g[:],
            qwd[g * QG * 128 : (g + 1) * QG * 128, :].rearrange(
                "(b p) c -> p b c", p=128
            ),
        )
        qw_gs.append(qw_g)

    def qw_ap(b):
        return qw_gs[b // QG][:, b % QG, :]

    w_ts = [
        wpool.tile([128, NS], F16, name=f"w{b}", tag="w") for b in range(KB)
    ]

    # x-transpose chunks for the wave phase, plus one prefetched for the
    # steady phase so the transition does not wait on the X-bar.
    xts = {}
    for c in range(NWCH + 1):
        r0 = c * TC
        xt = xtpool.tile([128, KB, TC], F16, name=f"xt{c}", tag="xt")
        nc.sync.dma_start_transpose(xt[:], xd[r0 : r0 + TC, :])
        xts[c] = xt

    # ---- dequant waves: unpack planes, apply (w4 - z) * s per segment.
    # Matmuls are b-major across all wave chunks so the PE tracks the
    # dequant stream in lockstep instead of stalling per chain.
    for si in range(3):
        off, sz = SEGS[si]
        p0, np_ = PLANES[si]
        pss = {}
        for c in range(NWCH):
            for tsub in range(TSUB):
                pss[(c, tsub)] = pspool.tile(
                    [128, sz], F32, name=f"ps_{c}_{tsub}_{si}", tag="ps"
                )
        for b in range(KB):
            stage = stpool.tile([128, np_ * CS], I32, name=f"st{si}_{b}", tag="st")
            for jj in range(np_):
                j = p0 + jj
                nc.vector.tensor_scalar(
                    stage[:, jj * CS : (jj + 1) * CS], qw_ap(b), 4 * j,
                    0xF, AOT.logical_shift_right, AOT.bitwise_and,
                )
            nc.scalar.copy(w_ts[b][:, p0 * CS : (p0 + np_) * CS], stage[:])
            zst = zpool.tile([128, 2 * sz], F16, name=f"zs{si}_{b}", tag="zs")
            nc.scalar.dma_start(
                zst[:], zsd[si][b * 128 : (b + 1) * 128, : 2 * sz]
            )
            wseg = w_ts[b][:, off : off + sz]
            nc.vector.tensor_tensor(wseg, wseg, zst[:, 0:sz], AOT.subtract)
            nc.vector.tensor_tensor(wseg, wseg, zst[:, sz : 2 * sz], AOT.mult)
            for c in range(NWCH):
                for tsub in range(TSUB):
                    st = xts[c][:, b, tsub * 128 : (tsub + 1) * 128]
                    nc.tensor.matmul(
                        pss[(c, tsub)][:], st, wseg,
                        start=(b == 0), stop=(b == KB - 1),
                    )
        for c in range(NWCH):
            for tsub in range(TSUB):
                ob = opool.tile(
                    [128, sz], F16, name=f"ob_{c}_{tsub}_{si}", tag="ob"
                )
                nc.vector.tensor_copy(ob[:], pss[(c, tsub)][:])
                ro = c * TC + tsub * 128
                nc.gpsimd.dma_start(
                    outd[ro : ro + 128, off : off + sz], ob[:]
                )

    # ---- steady chunks ----
    w_seg_slices = [
        [w_ts[b][:, off : off + sz] for b in range(KB)] for off, sz in SEGS
    ]
    for c in range(NWCH, NCH):
        r0 = c * TC
        if c in xts:
            xt = xts[c]
        else:
            xt = xtpool.tile([128, KB, TC], F16, name=f"xt{c}", tag="xt")
            nc.sync.dma_start_transpose(xt[:], xd[r0 : r0 + TC, :])
        for tsub in range(TSUB):
            pss = []
            for si, (off, sz) in enumerate(SEGS):
                ps = pspool.tile(
                    [128, sz], F32, name=f"ps_{c}_{tsub}_{si}", tag="ps"
                )
                pss.append(ps)
            for b in range(KB):
                st = xt[:, b, tsub * 128 : (tsub + 1) * 128]
                for si, (off, sz) in enumerate(SEGS):
                    nc.tensor.matmul(
                        pss[si][:],
                        st,
                        w_seg_slices[si][b],
                        start=(b == 0),
                        stop=(b == KB - 1),
                    )
            for si, (off, sz) in enumerate(SEGS):
                ob = opool.tile(
                    [128, sz], F16, name=f"ob_{c}_{tsub}_{si}", tag="ob"
                )
                nc.any.tensor_copy(ob[:], pss[si][:])
                ro = r0 + tsub * 128
                nc.gpsimd.dma_start(
                    outd[ro : ro + 128, off : off + sz], ob[:]
                )


def build_kernel():
    nc = bacc.Bacc("TRN2", target_bir_lowering=False, debug=False)
    xd = nc.dram_tensor("x", [T, K], F16, kind="ExternalInput").ap()
    qwd = nc.dram_tensor("qw", [K, CS], I32, kind="ExternalInput").ap()
    zsd = [
        nc.dram_tensor(
            f"zs{si}", [KB * 128, 2 * sz], F16, kind="ExternalInput"
        ).ap()
        for si, (off, sz) in enumerate(SEGS)
    ]
    outd = nc.dram_tensor("out", [T, NS], F16, kind="ExternalOutput").ap()
    with tile.TileContext(nc) as tc, ExitStack() as ctx:
        _body(ctx, tc, xd, qwd, zsd, outd)
    nc.compile()
    return nc


_NC = None


def _get_nc():
    global _NC
    if _NC is None:
        _NC = build_kernel()
    return _NC


# device col n' = j*CS + c  <->  logical col n = c*8 + j (nibble-plane-major)
_N = np.arange(NS)
_PERM = (_N % CS) * 8 + (_N // CS)  # logical col for each device col
_INV = (_N % 8) * CS + (_N // 8)  # device col for each logical col


def _unpack_u4(packed):
    shifts = np.arange(8, dtype=np.int32) * 4
    nib = (packed[:, :, None] >> shifts) & 0xF
    return nib.reshape(packed.shape[0], -1)


def make_in_maps(x, qweight, qzeros, scales):
    x = np.asarray(x, dtype=np.float16)
    qweight = np.asarray(qweight, dtype=np.int32)
    qzeros = np.asarray(qzeros, dtype=np.int32)
    scales = np.asarray(scales, dtype=np.float16)
    in_maps = []
    for c in range(NCORES):
        z_dev = _unpack_u4(qzeros[:, c * CS : (c + 1) * CS]).astype(np.float16)[
            :, _PERM
        ]
        s_dev = scales[:, c * NS : (c + 1) * NS][:, _PERM]
        m = {
            "x": x,
            "qw": np.ascontiguousarray(qweight[:, c * CS : (c + 1) * CS]),
        }
        for si, (off, sz) in enumerate(SEGS):
            zs = np.empty((KB, 128, 2, sz), dtype=np.float16)
            zs[:, :, 0, :] = z_dev[:, None, off : off + sz]
            zs[:, :, 1, :] = s_dev[:, None, off : off + sz]
            m[f"zs{si}"] = zs.reshape(KB * 128, 2 * sz)
        in_maps.append(m)
    return in_maps


def run(in_maps, **kwargs):
    return run_bass_kernel_spmd(
        _get_nc(), in_maps, core_ids=list(range(NCORES)), **kwargs
    )


def kernel(x, qweight, qzeros, scales):
    res = run(make_in_maps(x, qweight, qzeros, scales))
    outs = [res.results[c]["out"][:, _INV] for c in range(NCORES)]
    return np.concatenate(outs, axis=1)
